# revision 11
# baseline (speedup 1.0000x reference)
"""Trainium2 Bass kernel: full (non-causal) multi-head attention.

Problem: B=2, S=2048, H=16, D=64, fp32 in/out.
  out[b,q,h,:] = softmax(Q K^T / sqrt(D))[q,:] @ V   per (b,h)

Strategy: attention is independent per (batch, head) pair. There are
B*H = 32 pairs; shard 4 pairs to each of the 8 NeuronCores
(head-parallel => zero inter-core communication). All sharding /
layout packing happens host-side in numpy (not timed); the NEFF per
core computes 4 full attention heads.

Per-core layout (host-prepared, bf16):
  qt  [128, 2*2048]  partition p<64 -> pair 2g d=p ; p>=64 -> pair 2g+1
  kt  [128, 2*2048]  same packing (transposed: partition = head dim)
  v1  [128, 4*16*65] V tiles [kb][128 k, 64 d] + a ones column (col 64)
                     -> PV matmul also accumulates the softmax row-sums.
  out [128, 4*16*64] fp32, partition = q % 128 within each q-block.

Per (pair, k-block kb of 128 keys):
  S^T[kb] = (K^T tile).T @ Q^T       (PE, bf16, contract=64, row-tiled)
  P^T[kb] = exp(S^T[kb] * 1/8)       (ScalarE, PSUM->SBUF bf16)
  O[qb]  += P^T[kb][:,qb].T @ V1[kb] (PE, bf16, contract=128, accum PSUM)
Then O[:, :64] * 1/O[:, 64] (DVE reciprocal + per-partition scalar mul).
"""

import sys

if '/opt/trn_rl_repo' not in sys.path:
    sys.path.insert(0, '/opt/trn_rl_repo')

import numpy as np
import ml_dtypes

from concourse import bacc, tile, mybir
from concourse.bass_utils import run_bass_kernel_spmd

B, S, H, D = 2, 2048, 16, 64
N_CORES = 8
PAIRS = B * H              # 32 (b,h) pairs
PPC = PAIRS // N_CORES     # 4 pairs per core
NKB = S // 128             # 16 k-blocks
NQB = S // 128             # 16 q-blocks
SCALE = 1.0 / np.sqrt(D)   # 0.125

BF16 = mybir.dt.bfloat16
I16 = mybir.dt.int16
F32 = mybir.dt.float32

# k-blocks whose exp runs on VectorE via the Schraudolph int trick
# (bits16 = round(s * EXP_A + EXP_B) reinterpreted as bf16); the rest run
# exact exp on ScalarE. Splitting the 16.8M exps across both engines
# removes the single-engine throughput wall.
DVE_KBS = frozenset({1, 3, 5, 7, 9, 11, 13, 15})
EXP_A = 128 * SCALE * 1.4426950408889634
EXP_B = 128.0 * 127.0 - 7.5  # tuned for round-to-nearest f32->i16 convert


def _build_kernel(reps=1):
    """reps>1 repeats the whole computation in one NEFF (timing use only)."""
    nc = bacc.Bacc("TRN2", target_bir_lowering=False, debug=False,
                   num_devices=N_CORES)
    qt_ap = nc.dram_tensor("qt", [128, 2 * S], BF16, kind="ExternalInput").ap()
    kt_ap = nc.dram_tensor("kt", [128, 2 * S], BF16, kind="ExternalInput").ap()
    v1_ap = nc.dram_tensor("v1", [128, PPC * NKB * 65], BF16,
                           kind="ExternalInput").ap()
    out_ap = nc.dram_tensor("out", [128, PPC * NQB * 64], F32,
                            kind="ExternalOutput").ap()

    with tile.TileContext(nc) as tc:
        import contextlib
        with contextlib.ExitStack() as ctx:
            in_pool = ctx.enter_context(tc.tile_pool(name="inp", bufs=1))
            pt_pool = ctx.enter_context(tc.tile_pool(name="pt", bufs=36))
            osb_pool = ctx.enter_context(tc.tile_pool(name="osb", bufs=2))
            rec_pool = ctx.enter_context(tc.tile_pool(name="rec", bufs=2))
            st_pool = ctx.enter_context(
                tc.tile_pool(name="st", bufs=1, space="PSUM"))
            o_pool = ctx.enter_context(
                tc.tile_pool(name="o", bufs=4, space="PSUM"))

            qt_sb = in_pool.tile([128, 2 * S], BF16)
            kt_sb = in_pool.tile([128, 2 * S], BF16)
            v1_sb = in_pool.tile([128, PPC * NKB * 65], BF16)
            nc.sync.dma_start(out=qt_sb[:], in_=qt_ap[:])
            nc.sync.dma_start(out=kt_sb[:], in_=kt_ap[:])
            nc.sync.dma_start(out=v1_sb[:], in_=v1_ap[:])

            def emit_exp(pt, st, kb, half):
                """exp of one [128,1024] S^T chunk: ScalarE (exact) or
                VectorE (Schraudolph int16 bit trick), split by k-block."""
                dst = pt[:, 1024 * half: 1024 * half + 1024]
                if kb in DVE_KBS:
                    nc.vector.tensor_scalar(
                        out=dst.bitcast(I16), in0=st[:],
                        scalar1=float(EXP_A), scalar2=float(EXP_B),
                        op0=mybir.AluOpType.mult, op1=mybir.AluOpType.add)
                else:
                    nc.scalar.activation(
                        dst, st[:], mybir.ActivationFunctionType.Exp,
                        scale=float(SCALE))

            def emit_pv_pair(rep, g, h, pts):
                """PV + normalize + store for pair p = 2g+h."""
                p = 2 * g + h
                osb = osb_pool.tile([128, NQB * 64], F32,
                                    name=f"osb_{rep}_{p}", tag="osb")
                for bt in range(4):
                    o4 = [o_pool.tile([128, 65], F32, tag="o",
                                      name=f"o_{rep}_{p}_{bt}_{i}")
                          for i in range(4)]
                    for kb in range(NKB):
                        vt = v1_sb[:, 1040 * p + 65 * kb:
                                   1040 * p + 65 * kb + 65]
                        for jj in range(4):
                            qb = 4 * bt + jj
                            nc.tensor.matmul(
                                o4[jj][:],
                                lhsT=pts[kb][:, 128 * qb: 128 * qb + 128],
                                rhs=vt,
                                start=(kb == 0), stop=(kb == NKB - 1),
                                skip_group_check=True)
                    # normalize: out[:, q, d] = o[:, q, d] / o[:, q, 64]
                    rec = rec_pool.tile([128, 4], F32, name=f"rec_{rep}_{p}_{bt}",
                                        tag="rec")
                    for jj in range(4):
                        nc.vector.reciprocal(rec[:, jj: jj + 1],
                                             o4[jj][:, 64: 65])
                    for jj in range(4):
                        qb = 4 * bt + jj
                        nc.vector.tensor_scalar_mul(
                            osb[:, 64 * qb: 64 * qb + 64],
                            o4[jj][:, 0: 64],
                            rec[:, jj: jj + 1])
                nc.sync.dma_start(
                    out=out_ap[:, 1024 * p: 1024 * p + 1024], in_=osb[:])

            for rep in range(reps):
                for g in range(2):
                    gq = 2048 * g
                    # --- S^T + exp, both head-streams interleaved per kb so
                    # the two contract=64 QK matmuls land on complementary
                    # halves of the PE array (row-tiled, run concurrently) ---
                    pts = {}
                    for kb in range(NKB):
                        for half in range(2):
                            for h in range(2):
                                if half == 0:
                                    pts[(h, kb)] = pt_pool.tile(
                                        [128, S], BF16,
                                        name=f"pt_{rep}_{g}_{h}_{kb}", tag="pt")
                                hs = slice(64 * h, 64 * h + 64)
                                ktile = kt_sb[hs, gq + 128 * kb:
                                              gq + 128 * kb + 128]
                                st = st_pool.tile([128, 1024], F32,
                                                  name=f"st_{rep}_{g}_{h}_{kb}_{half}",
                                                  tag=f"st{h}")
                                for j in range(2):
                                    q0 = gq + 1024 * half + 512 * j
                                    nc.tensor.matmul(
                                        st[:, 512 * j: 512 * j + 512],
                                        lhsT=ktile,
                                        rhs=qt_sb[hs, q0: q0 + 512],
                                        start=True, stop=True)
                                emit_exp(pts[(h, kb)], st, kb, half)
                    for h in range(2):
                        emit_pv_pair(rep, g, h, [pts[(h, kb)]
                                                 for kb in range(NKB)])

    nc.compile()
    return nc


_NC_CACHE = {}


def _get_nc(reps=1):
    key = ("nc", reps)
    if key not in _NC_CACHE:
        _NC_CACHE[key] = _build_kernel(reps)
    return _NC_CACHE[key]


def _shard_inputs(query, key, value):
    """Full [B,S,H,D] f32 -> per-core bf16 packed arrays."""
    bf = ml_dtypes.bfloat16
    # [B,S,H,D] -> [B,H,S,D] -> [32, S, D]
    q = np.ascontiguousarray(query.transpose(0, 2, 1, 3)).reshape(PAIRS, S, D)
    k = np.ascontiguousarray(key.transpose(0, 2, 1, 3)).reshape(PAIRS, S, D)
    v = np.ascontiguousarray(value.transpose(0, 2, 1, 3)).reshape(PAIRS, S, D)
    in_maps = []
    for c in range(N_CORES):
        sl = slice(PPC * c, PPC * (c + 1))
        qc, kc, vc = q[sl], k[sl], v[sl]
        # transposed: [4, S, D] -> [4, D, S] -> [2, 128, S] -> [128, 2*S]
        qt = qc.transpose(0, 2, 1).reshape(2, 128, S).transpose(1, 0, 2) \
            .reshape(128, 2 * S)
        kt = kc.transpose(0, 2, 1).reshape(2, 128, S).transpose(1, 0, 2) \
            .reshape(128, 2 * S)
        # v: [4, S, D] -> [4, 16, 128, D] -> ones col -> [128, 4*16*65]
        v4 = vc.reshape(PPC, NKB, 128, D)
        v1 = np.ones((PPC, NKB, 128, D + 1), np.float32)
        v1[:, :, :, :D] = v4
        v1 = v1.transpose(2, 0, 1, 3).reshape(128, PPC * NKB * 65)
        in_maps.append({
            "qt": np.ascontiguousarray(qt).astype(bf),
            "kt": np.ascontiguousarray(kt).astype(bf),
            "v1": np.ascontiguousarray(v1).astype(bf),
        })
    return in_maps


def _unshard_output(results):
    """Per-core out [128, 4*16*64] f32 -> full [B,S,H,D] f32."""
    outs = []
    for c in range(N_CORES):
        o = results[c]["out"].reshape(128, PPC, NQB, D)
        outs.append(o.transpose(1, 2, 0, 3).reshape(PPC, S, D))
    full = np.concatenate(outs, axis=0)          # [32, S, D]
    full = full.reshape(B, H, S, D).transpose(0, 2, 1, 3)  # [B,S,H,D]
    return np.ascontiguousarray(full)


def kernel(query, key, value):
    nc = _get_nc()
    in_maps = _shard_inputs(np.asarray(query, np.float32),
                            np.asarray(key, np.float32),
                            np.asarray(value, np.float32))
    res = run_bass_kernel_spmd(nc, in_maps, core_ids=list(range(N_CORES)))
    return _unshard_output(res.results)


if __name__ == "__main__":
    rng = np.random.default_rng(0)
    q = rng.standard_normal((B, S, H, D), np.float32)
    k = rng.standard_normal((B, S, H, D), np.float32)
    v = rng.standard_normal((B, S, H, D), np.float32)
    o = kernel(query=q, key=k, value=v)
    print("out", o.shape, o.dtype, np.abs(o).mean())


# revision 14
# speedup vs baseline: 1.6818x; 1.6818x over previous
"""Trainium2 Bass kernel: full (non-causal) multi-head attention.

Problem: B=2, S=2048, H=16, D=64, fp32 in/out.
  out[b,q,h,:] = softmax(Q K^T / sqrt(D))[q,:] @ V   per (b,h)

Strategy: attention is independent per (batch, head) pair. There are
B*H = 32 pairs; shard 4 pairs to each of the 8 NeuronCores
(head-parallel => zero inter-core communication). All sharding /
layout packing happens host-side in numpy (not timed); the NEFF per
core computes 4 full attention heads.

Per-core layout (host-prepared, bf16):
  qt  [128, 2*2048]  partition p<64 -> pair 2g d=p ; p>=64 -> pair 2g+1
  kt  [128, 2*2048]  same packing (transposed: partition = head dim)
  v1  [128, 4*16*65] V tiles [kb][128 k, 64 d] + a ones column (col 64)
                     -> PV matmul also accumulates the softmax row-sums.
  out [128, 4*16*64] fp32, partition = q % 128 within each q-block.

Per (pair, k-block kb of 128 keys):
  S^T[kb] = (K^T tile).T @ Q^T       (PE, bf16, contract=64, row-tiled)
  P^T[kb] = exp(S^T[kb] * 1/8)       (ScalarE, PSUM->SBUF bf16)
  O[qb]  += P^T[kb][:,qb].T @ V1[kb] (PE, bf16, contract=128, accum PSUM)
Then O[:, :64] * 1/O[:, 64] (DVE reciprocal + per-partition scalar mul).
"""

import sys

if '/opt/trn_rl_repo' not in sys.path:
    sys.path.insert(0, '/opt/trn_rl_repo')

import numpy as np
import ml_dtypes

from concourse import bacc, tile, mybir
from concourse.bass_utils import run_bass_kernel_spmd

B, S, H, D = 2, 2048, 16, 64
N_CORES = 8
PAIRS = B * H              # 32 (b,h) pairs
PPC = PAIRS // N_CORES     # 4 pairs per core
NKB = S // 128             # 16 k-blocks
NQB = S // 128             # 16 q-blocks
SCALE = 1.0 / np.sqrt(D)   # 0.125

BF16 = mybir.dt.bfloat16
I16 = mybir.dt.int16
F32 = mybir.dt.float32

# exp runs on VectorE via the Schraudolph int trick for half the chunks
# (bits16 = round(s * EXP_A + EXP_B) reinterpreted as bf16); the rest run
# exact exp on ScalarE. Alternating by (kb + h) parity keeps BOTH engines
# busy at all times (the two head-streams are always on different engines)
# while each head still mixes exact/approx 50/50 across its k-blocks.
EXP_A = 128 * SCALE * 1.4426950408889634
EXP_B = 128.0 * 127.0 - 7.5  # tuned for round-to-nearest f32->i16 convert


def _use_dve(h, kb):
    return (kb + h) % 2 == 1


def _build_kernel(reps=1):
    """reps>1 repeats the whole computation in one NEFF (timing use only)."""
    nc = bacc.Bacc("TRN2", target_bir_lowering=False, debug=False,
                   num_devices=N_CORES)
    qt_ap = nc.dram_tensor("qt", [128, 2 * S], BF16, kind="ExternalInput").ap()
    kt_ap = nc.dram_tensor("kt", [128, 2 * S], BF16, kind="ExternalInput").ap()
    v1_ap = nc.dram_tensor("v1", [128, PPC * NKB * 65], BF16,
                           kind="ExternalInput").ap()
    out_ap = nc.dram_tensor("out", [128, PPC * NQB * 64], F32,
                            kind="ExternalOutput").ap()

    with tile.TileContext(nc) as tc:
        import contextlib
        with contextlib.ExitStack() as ctx:
            in_pool = ctx.enter_context(tc.tile_pool(name="inp", bufs=1))
            pt_pool = ctx.enter_context(tc.tile_pool(name="pt", bufs=36))
            osb_pool = ctx.enter_context(tc.tile_pool(name="osb", bufs=2))
            rec_pool = ctx.enter_context(tc.tile_pool(name="rec", bufs=2))
            st_pool = ctx.enter_context(
                tc.tile_pool(name="st", bufs=1, space="PSUM"))
            o_pool = ctx.enter_context(
                tc.tile_pool(name="o", bufs=4, space="PSUM"))

            qt_sb = in_pool.tile([128, 2 * S], BF16)
            kt_sb = in_pool.tile([128, 2 * S], BF16)
            v1_sb = in_pool.tile([128, PPC * NKB * 65], BF16)
            nc.sync.dma_start(out=qt_sb[:], in_=qt_ap[:])
            nc.sync.dma_start(out=kt_sb[:], in_=kt_ap[:])
            nc.sync.dma_start(out=v1_sb[:], in_=v1_ap[:])

            def emit_exp(pt, st, h, kb, half):
                """exp of one [128,1024] S^T chunk: ScalarE (exact) or
                VectorE (Schraudolph int16 bit trick)."""
                dst = pt[:, 1024 * half: 1024 * half + 1024]
                if _use_dve(h, kb):
                    nc.vector.tensor_scalar(
                        out=dst.bitcast(I16), in0=st[:],
                        scalar1=float(EXP_A), scalar2=float(EXP_B),
                        op0=mybir.AluOpType.mult, op1=mybir.AluOpType.add)
                else:
                    nc.scalar.activation(
                        dst, st[:], mybir.ActivationFunctionType.Exp,
                        scale=float(SCALE))

            def emit_pv_pair(rep, g, h, pts):
                """PV + normalize + store for pair p = 2g+h."""
                p = 2 * g + h
                osb = osb_pool.tile([128, NQB * 64], F32,
                                    name=f"osb_{rep}_{p}", tag="osb")
                for bt in range(4):
                    o4 = [o_pool.tile([128, 65], F32, tag="o",
                                      name=f"o_{rep}_{p}_{bt}_{i}")
                          for i in range(4)]
                    for kb in range(NKB):
                        vt = v1_sb[:, 1040 * p + 65 * kb:
                                   1040 * p + 65 * kb + 65]
                        for jj in range(4):
                            qb = 4 * bt + jj
                            nc.tensor.matmul(
                                o4[jj][:],
                                lhsT=pts[kb][:, 128 * qb: 128 * qb + 128],
                                rhs=vt,
                                start=(kb == 0), stop=(kb == NKB - 1),
                                skip_group_check=True)
                    # normalize: out[:, q, d] = o[:, q, d] / o[:, q, 64]
                    rec = rec_pool.tile([128, 4], F32, name=f"rec_{rep}_{p}_{bt}",
                                        tag="rec")
                    for jj in range(4):
                        nc.vector.reciprocal(rec[:, jj: jj + 1],
                                             o4[jj][:, 64: 65])
                    for jj in range(4):
                        qb = 4 * bt + jj
                        nc.vector.tensor_scalar_mul(
                            osb[:, 64 * qb: 64 * qb + 64],
                            o4[jj][:, 0: 64],
                            rec[:, jj: jj + 1])
                nc.sync.dma_start(
                    out=out_ap[:, 1024 * p: 1024 * p + 1024], in_=osb[:])

            for rep in range(reps):
                for g in range(2):
                    gq = 2048 * g
                    # --- S^T + exp, both head-streams interleaved per kb so
                    # the two contract=64 QK matmuls land on complementary
                    # halves of the PE array (row-tiled, run concurrently) ---
                    pts = {}
                    for kb in range(NKB):
                        for half in range(2):
                            for h in range(2):
                                if half == 0:
                                    pts[(h, kb)] = pt_pool.tile(
                                        [128, S], BF16,
                                        name=f"pt_{rep}_{g}_{h}_{kb}", tag="pt")
                                hs = slice(64 * h, 64 * h + 64)
                                ktile = kt_sb[hs, gq + 128 * kb:
                                              gq + 128 * kb + 128]
                                st = st_pool.tile([128, 1024], F32,
                                                  name=f"st_{rep}_{g}_{h}_{kb}_{half}",
                                                  tag=f"st{h}")
                                for j in range(2):
                                    q0 = gq + 1024 * half + 512 * j
                                    nc.tensor.matmul(
                                        st[:, 512 * j: 512 * j + 512],
                                        lhsT=ktile,
                                        rhs=qt_sb[hs, q0: q0 + 512],
                                        start=True, stop=True)
                                emit_exp(pts[(h, kb)], st, h, kb, half)
                    for h in range(2):
                        emit_pv_pair(rep, g, h, [pts[(h, kb)]
                                                 for kb in range(NKB)])

    nc.compile()
    return nc


_NC_CACHE = {}


def _get_nc(reps=1):
    key = ("nc", reps)
    if key not in _NC_CACHE:
        _NC_CACHE[key] = _build_kernel(reps)
    return _NC_CACHE[key]


def _shard_inputs(query, key, value):
    """Full [B,S,H,D] f32 -> per-core bf16 packed arrays."""
    bf = ml_dtypes.bfloat16
    # [B,S,H,D] -> [B,H,S,D] -> [32, S, D]
    q = np.ascontiguousarray(query.transpose(0, 2, 1, 3)).reshape(PAIRS, S, D)
    k = np.ascontiguousarray(key.transpose(0, 2, 1, 3)).reshape(PAIRS, S, D)
    v = np.ascontiguousarray(value.transpose(0, 2, 1, 3)).reshape(PAIRS, S, D)
    in_maps = []
    for c in range(N_CORES):
        sl = slice(PPC * c, PPC * (c + 1))
        qc, kc, vc = q[sl], k[sl], v[sl]
        # transposed: [4, S, D] -> [4, D, S] -> [2, 128, S] -> [128, 2*S]
        qt = qc.transpose(0, 2, 1).reshape(2, 128, S).transpose(1, 0, 2) \
            .reshape(128, 2 * S)
        kt = kc.transpose(0, 2, 1).reshape(2, 128, S).transpose(1, 0, 2) \
            .reshape(128, 2 * S)
        # v: [4, S, D] -> [4, 16, 128, D] -> ones col -> [128, 4*16*65]
        v4 = vc.reshape(PPC, NKB, 128, D)
        v1 = np.ones((PPC, NKB, 128, D + 1), np.float32)
        v1[:, :, :, :D] = v4
        v1 = v1.transpose(2, 0, 1, 3).reshape(128, PPC * NKB * 65)
        in_maps.append({
            "qt": np.ascontiguousarray(qt).astype(bf),
            "kt": np.ascontiguousarray(kt).astype(bf),
            "v1": np.ascontiguousarray(v1).astype(bf),
        })
    return in_maps


def _unshard_output(results):
    """Per-core out [128, 4*16*64] f32 -> full [B,S,H,D] f32."""
    outs = []
    for c in range(N_CORES):
        o = results[c]["out"].reshape(128, PPC, NQB, D)
        outs.append(o.transpose(1, 2, 0, 3).reshape(PPC, S, D))
    full = np.concatenate(outs, axis=0)          # [32, S, D]
    full = full.reshape(B, H, S, D).transpose(0, 2, 1, 3)  # [B,S,H,D]
    return np.ascontiguousarray(full)


def kernel(query, key, value):
    nc = _get_nc()
    in_maps = _shard_inputs(np.asarray(query, np.float32),
                            np.asarray(key, np.float32),
                            np.asarray(value, np.float32))
    res = run_bass_kernel_spmd(nc, in_maps, core_ids=list(range(N_CORES)))
    return _unshard_output(res.results)


if __name__ == "__main__":
    rng = np.random.default_rng(0)
    q = rng.standard_normal((B, S, H, D), np.float32)
    k = rng.standard_normal((B, S, H, D), np.float32)
    v = rng.standard_normal((B, S, H, D), np.float32)
    o = kernel(query=q, key=k, value=v)
    print("out", o.shape, o.dtype, np.abs(o).mean())


# revision 16
# speedup vs baseline: 1.7379x; 1.0334x over previous
"""Trainium2 Bass kernel: full (non-causal) multi-head attention.

Problem: B=2, S=2048, H=16, D=64, fp32 in/out.
  out[b,q,h,:] = softmax(Q K^T / sqrt(D))[q,:] @ V   per (b,h)

Strategy: attention is independent per (batch, head) pair. There are
B*H = 32 pairs; shard 4 pairs to each of the 8 NeuronCores
(head-parallel => zero inter-core communication). All sharding /
layout packing happens host-side in numpy (not timed); the NEFF per
core computes 4 full attention heads.

Per-core layout (host-prepared, bf16):
  qt  [128, 2*2048]  partition p<64 -> pair 2g d=p ; p>=64 -> pair 2g+1
  kt  [128, 2*2048]  same packing (transposed: partition = head dim)
  v1  [128, 4*16*65] V tiles [kb][128 k, 64 d] + a ones column (col 64)
                     -> PV matmul also accumulates the softmax row-sums.
  out [128, 4*16*64] fp32, partition = q % 128 within each q-block.

Per (pair, k-block kb of 128 keys):
  S^T[kb] = (K^T tile).T @ Q^T       (PE, bf16, contract=64, row-tiled)
  P^T[kb] = exp(S^T[kb] * 1/8)       (ScalarE, PSUM->SBUF bf16)
  O[qb]  += P^T[kb][:,qb].T @ V1[kb] (PE, bf16, contract=128, accum PSUM)
Then O[:, :64] * 1/O[:, 64] (DVE reciprocal + per-partition scalar mul).
"""

import sys

if '/opt/trn_rl_repo' not in sys.path:
    sys.path.insert(0, '/opt/trn_rl_repo')

import numpy as np
import ml_dtypes

from concourse import bacc, tile, mybir
from concourse.bass_utils import run_bass_kernel_spmd

B, S, H, D = 2, 2048, 16, 64
N_CORES = 8
PAIRS = B * H              # 32 (b,h) pairs
PPC = PAIRS // N_CORES     # 4 pairs per core
NKB = S // 128             # 16 k-blocks
NQB = S // 128             # 16 q-blocks
SCALE = 1.0 / np.sqrt(D)   # 0.125

BF16 = mybir.dt.bfloat16
I16 = mybir.dt.int16
F32 = mybir.dt.float32

# exp runs on VectorE via the Schraudolph int trick for half the chunks
# (bits16 = round(s * EXP_A + EXP_B) reinterpreted as bf16); the rest run
# exact exp on ScalarE. Alternating by (kb + h) parity keeps BOTH engines
# busy at all times (the two head-streams are always on different engines)
# while each head still mixes exact/approx 50/50 across its k-blocks.
EXP_A = 128 * SCALE * 1.4426950408889634
EXP_B = 128.0 * 127.0 - 7.5  # tuned for round-to-nearest f32->i16 convert


def _use_dve(h, kb):
    return (kb + h) % 2 == 1


def _build_kernel(reps=1):
    """reps>1 repeats the whole computation in one NEFF (timing use only)."""
    nc = bacc.Bacc("TRN2", target_bir_lowering=False, debug=False,
                   num_devices=N_CORES)
    qt_ap = nc.dram_tensor("qt", [128, 2 * S], BF16, kind="ExternalInput").ap()
    kt_ap = nc.dram_tensor("kt", [128, 2 * S], BF16, kind="ExternalInput").ap()
    v1_ap = nc.dram_tensor("v1", [128, PPC * NKB * 65], BF16,
                           kind="ExternalInput").ap()
    out_ap = nc.dram_tensor("out", [128, PPC * NQB * 64], F32,
                            kind="ExternalOutput").ap()

    with tile.TileContext(nc) as tc:
        import contextlib
        with contextlib.ExitStack() as ctx:
            in_pool = ctx.enter_context(tc.tile_pool(name="inp", bufs=1))
            pt_pool = ctx.enter_context(tc.tile_pool(name="pt", bufs=4))
            osb_pool = ctx.enter_context(tc.tile_pool(name="osb", bufs=2))
            rec_pool = ctx.enter_context(tc.tile_pool(name="rec", bufs=2))
            st_pool = ctx.enter_context(
                tc.tile_pool(name="st", bufs=2, space="PSUM"))
            o_pool = ctx.enter_context(
                tc.tile_pool(name="o", bufs=4, space="PSUM"))

            qt_sb = in_pool.tile([128, 2 * S], BF16)
            kt_sb = in_pool.tile([128, 2 * S], BF16)
            v1_sb = in_pool.tile([128, PPC * NKB * 65], BF16)
            nc.sync.dma_start(out=qt_sb[:], in_=qt_ap[:])
            nc.sync.dma_start(out=kt_sb[:], in_=kt_ap[:])
            nc.sync.dma_start(out=v1_sb[:], in_=v1_ap[:])

            def emit_exp(pt, st, half):
                """exp of one [128,1024] S^T chunk: even halves exact on
                ScalarE, odd halves VectorE Schraudolph int16 bit trick —
                both engines stay busy concurrently, each head's probs mix
                exact/approx 50/50."""
                dst = pt[:, 1024 * half: 1024 * half + 1024]
                if half % 2 == 1:
                    nc.vector.tensor_scalar(
                        out=dst.bitcast(I16), in0=st[:],
                        scalar1=float(EXP_A), scalar2=float(EXP_B),
                        op0=mybir.AluOpType.mult, op1=mybir.AluOpType.add)
                else:
                    nc.scalar.activation(
                        dst, st[:], mybir.ActivationFunctionType.Exp,
                        scale=float(SCALE))

            for rep in range(reps):
              for p in range(PPC):
                g, h = p // 2, p % 2
                hs = slice(64 * h, 64 * h + 64)
                gq = 2048 * g

                # 4 PSUM accumulator banks, 4 q-blocks each. start=True
                # zeroes the whole 2 KiB bank region, so ONLY the first
                # sub-slice's first matmul starts the group; siblings
                # accumulate onto the zeroed region.
                o4 = [o_pool.tile([128, 4 * 65], F32, tag="o4",
                                  name=f"o4_{rep}_{p}_{i}")
                      for i in range(4)]

                for kb in range(NKB):
                    ktile = kt_sb[hs, gq + 128 * kb: gq + 128 * kb + 128]
                    pt = pt_pool.tile([128, S], BF16, name=f"pt_{rep}_{p}_{kb}",
                                      tag="pt")
                    for half in range(2):
                        st = st_pool.tile([128, 1024], F32,
                                          name=f"st_{rep}_{p}_{kb}_{half}",
                                          tag="st")
                        for j in range(2):
                            q0 = gq + 1024 * half + 512 * j
                            nc.tensor.matmul(
                                st[:, 512 * j: 512 * j + 512],
                                lhsT=ktile,
                                rhs=qt_sb[hs, q0: q0 + 512],
                                start=True, stop=True)
                        emit_exp(pt, st, half)
                    vt = v1_sb[:, 1040 * p + 65 * kb: 1040 * p + 65 * kb + 65]
                    for qb in range(NQB):
                        nc.tensor.matmul(
                            o4[qb // 4][:, 65 * (qb % 4): 65 * (qb % 4) + 65],
                            lhsT=pt[:, 128 * qb: 128 * qb + 128],
                            rhs=vt,
                            start=(kb == 0 and qb % 4 == 0),
                            stop=(kb == NKB - 1),
                            skip_group_check=True)

                # normalize: out[:, q, d] = o[:, q, d] / o[:, q, 64]
                osb = osb_pool.tile([128, NQB * 64], F32,
                                    name=f"osb_{rep}_{p}", tag="osb")
                rec = rec_pool.tile([128, NQB], F32, name=f"rec_{rep}_{p}",
                                    tag="rec")
                for j4 in range(4):
                    sums = o4[j4][:].rearrange("p (j c) -> p j c", c=65)[:, :, 64]
                    nc.vector.reciprocal(rec[:, 4 * j4: 4 * j4 + 4], sums)
                for qb in range(NQB):
                    nc.vector.tensor_scalar_mul(
                        osb[:, 64 * qb: 64 * qb + 64],
                        o4[qb // 4][:, 65 * (qb % 4): 65 * (qb % 4) + 64],
                        rec[:, qb: qb + 1])
                nc.sync.dma_start(
                    out=out_ap[:, 1024 * p: 1024 * p + 1024], in_=osb[:])

    nc.compile()
    return nc


_NC_CACHE = {}


def _get_nc(reps=1):
    key = ("nc", reps)
    if key not in _NC_CACHE:
        _NC_CACHE[key] = _build_kernel(reps)
    return _NC_CACHE[key]


def _shard_inputs(query, key, value):
    """Full [B,S,H,D] f32 -> per-core bf16 packed arrays."""
    bf = ml_dtypes.bfloat16
    # [B,S,H,D] -> [B,H,S,D] -> [32, S, D]
    q = np.ascontiguousarray(query.transpose(0, 2, 1, 3)).reshape(PAIRS, S, D)
    k = np.ascontiguousarray(key.transpose(0, 2, 1, 3)).reshape(PAIRS, S, D)
    v = np.ascontiguousarray(value.transpose(0, 2, 1, 3)).reshape(PAIRS, S, D)
    in_maps = []
    for c in range(N_CORES):
        sl = slice(PPC * c, PPC * (c + 1))
        qc, kc, vc = q[sl], k[sl], v[sl]
        # transposed: [4, S, D] -> [4, D, S] -> [2, 128, S] -> [128, 2*S]
        qt = qc.transpose(0, 2, 1).reshape(2, 128, S).transpose(1, 0, 2) \
            .reshape(128, 2 * S)
        kt = kc.transpose(0, 2, 1).reshape(2, 128, S).transpose(1, 0, 2) \
            .reshape(128, 2 * S)
        # v: [4, S, D] -> [4, 16, 128, D] -> ones col -> [128, 4*16*65]
        v4 = vc.reshape(PPC, NKB, 128, D)
        v1 = np.ones((PPC, NKB, 128, D + 1), np.float32)
        v1[:, :, :, :D] = v4
        v1 = v1.transpose(2, 0, 1, 3).reshape(128, PPC * NKB * 65)
        in_maps.append({
            "qt": np.ascontiguousarray(qt).astype(bf),
            "kt": np.ascontiguousarray(kt).astype(bf),
            "v1": np.ascontiguousarray(v1).astype(bf),
        })
    return in_maps


def _unshard_output(results):
    """Per-core out [128, 4*16*64] f32 -> full [B,S,H,D] f32."""
    outs = []
    for c in range(N_CORES):
        o = results[c]["out"].reshape(128, PPC, NQB, D)
        outs.append(o.transpose(1, 2, 0, 3).reshape(PPC, S, D))
    full = np.concatenate(outs, axis=0)          # [32, S, D]
    full = full.reshape(B, H, S, D).transpose(0, 2, 1, 3)  # [B,S,H,D]
    return np.ascontiguousarray(full)


def kernel(query, key, value):
    nc = _get_nc()
    in_maps = _shard_inputs(np.asarray(query, np.float32),
                            np.asarray(key, np.float32),
                            np.asarray(value, np.float32))
    res = run_bass_kernel_spmd(nc, in_maps, core_ids=list(range(N_CORES)))
    return _unshard_output(res.results)


if __name__ == "__main__":
    rng = np.random.default_rng(0)
    q = rng.standard_normal((B, S, H, D), np.float32)
    k = rng.standard_normal((B, S, H, D), np.float32)
    v = rng.standard_normal((B, S, H, D), np.float32)
    o = kernel(query=q, key=k, value=v)
    print("out", o.shape, o.dtype, np.abs(o).mean())
